# revision 3
# baseline (speedup 1.0000x reference)
"""Multi-head causal attention (B=4, T=2048, C=1024, H=16, DH=64) on 8 trn2
NeuronCores.

Sharding: core = (batch, head-half): core 2*b+g computes heads g*8..g*8+8 of
batch b, including the partial output projection with the matching 512 rows
of Wp (tensor-parallel). Host-side unshard sums the two partials per batch
and adds bp.

v2 dtype strategy (tolerance 2e-2, baseline fp32r ~2e-4):
  - Q/K projections: fp8(e4m3) DoubleRow matmuls (0.5 cyc/row), weights
    host-scaled x16 so W entries sit in e4m3 normal range; the resulting
    x256 logit scale is folded into the softmax exp scale (1/8192).
  - QK logit matmuls: plain fp8 (1 cyc/row); q/k quantization error enters
    logits through the tiny softmax scale -> ~0.5% P error.
  - V projection / AV / output projection: bf16 (1 cyc/row at any width;
    fp32r pays 4x below 256-wide).  P is exp()'d to bf16.
  - Out DMA'd as bf16 partials, summed in f32 host-side (+bp).

Engine placement: exp (softmax) is the critical engine (~116us busy on ACT),
so PSUM->SBUF copies move off it: q/k fp8 copies on ACT in phase 1 (idle
then), v/o_stage/recip/ob on DVE, normalize multiply on GPSIMD.

Unit order is tj-major so phase-1 projections (tn-ordered) feed the first
attention units ASAP and the output projection for tj overlaps tj+1.
"""
import numpy as np
import ml_dtypes

import concourse.mybir as mybir
import concourse.tile as tile
from concourse import bacc, bass_utils

F32 = mybir.dt.float32
BF16 = mybir.dt.bfloat16
F8 = mybir.dt.float8e4
DR = mybir.MatmulPerfMode.DoubleRow

B, T, C, H, DH = 4, 2048, 1024, 16, 64
HG = H // 2          # heads per core (8)
CC = C // 128        # contraction chunks (8)
CP = CC // 2         # DoubleRow contraction pairs (4)
TJ = 512             # query chunk width
NTJ = T // TJ        # 4
NSI = T // 128       # 16 key chunks
MASK_VAL = -1.0e6
W_SCALE = 16.0       # host scale on Wq/Wk for fp8 range
SCALE = 1.0 / (32.0 * W_SCALE * W_SCALE)  # 1/sqrt(C) / (16*16)

NP_F8 = ml_dtypes.float8_e4m3
NP_BF16 = ml_dtypes.bfloat16

TRACE = False
_NC_CACHE = {}


def _build():
    nc = bacc.Bacc(trn_type="TRN2", target_bir_lowering=False, debug=False)

    xq = nc.dram_tensor("xq", [CP, 128, 2, T], F8, kind="ExternalInput")
    xv = nc.dram_tensor("xv", [C, T], BF16, kind="ExternalInput")
    wqk = nc.dram_tensor("wqk", [CP, 128, 2, 2 * HG * DH], F8,
                         kind="ExternalInput")
    wv = nc.dram_tensor("wv", [C, HG * DH], BF16, kind="ExternalInput")
    wp = nc.dram_tensor("wp", [HG * DH, C], BF16, kind="ExternalInput")
    tril = nc.dram_tensor("tril", [128, 128], F32, kind="ExternalInput")
    ones8 = nc.dram_tensor("ones8", [128, HG], BF16, kind="ExternalInput")
    zrow = nc.dram_tensor("zrow", [1, TJ], F8, kind="ExternalInput")
    out = nc.dram_tensor("out", [T, C], BF16, kind="ExternalOutput")

    q_dram = nc.dram_tensor("q_scratch", [HG * DH, T], F8)
    l_dram = nc.dram_tensor("l_scratch", [HG * NTJ, TJ], F32)

    with tile.TileContext(nc) as tc:
        with (
            tc.tile_pool(name="persist", bufs=1) as persist,
            tc.tile_pool(name="qstage", bufs=5) as qstage,
        ):
            tril_sb = persist.tile([128, 128], F32)
            nc.sync.dma_start(out=tril_sb, in_=tril.ap())
            ones_sb = persist.tile([128, HG], BF16)
            nc.sync.dma_start(out=ones_sb, in_=ones8.ap())

            k_sb = [persist.tile([128, T], F8, name=f"k_{mg}")
                    for mg in range(HG // 2)]
            # zero-padded Q operands: the other head's 64 rows are zeros so a
            # full-128-contraction matmul computes exactly one head.
            q0pad = [persist.tile([128, TJ], F8, name=f"q0pad_{i}")
                     for i in range(2)]
            q1pad = [persist.tile([128, TJ], F8, name=f"q1pad_{i}")
                     for i in range(2)]
            for i in range(2):
                nc.sync.dma_start(out=q0pad[i][64:128, :],
                                  in_=zrow.ap().to_broadcast((64, TJ)))
                nc.sync.dma_start(out=q1pad[i][0:64, :],
                                  in_=zrow.ap().to_broadcast((64, TJ)))
            v_sb = [persist.tile([128, HG, DH + 1], BF16, name=f"v_{si}")
                    for si in range(NSI)]

            # ---- phase 1: projections ----
            with (
                tc.tile_pool(name="xw", bufs=1) as xw,
                tc.tile_pool(name="pps", bufs=8, space="PSUM") as pps,
            ):
                wqk_sb = [xw.tile([128, 2, 2 * HG * DH], F8, name=f"wqk_{j}")
                          for j in range(CP)]
                xq_sb = [xw.tile([128, 2, T], F8, name=f"xq_{j}")
                         for j in range(CP)]
                wv_sb = [xw.tile([128, HG * DH], BF16, name=f"wv_{c}")
                         for c in range(CC)]
                xv_sb = [xw.tile([128, T], BF16, name=f"xv_{c}")
                         for c in range(CC)]

                # DMA in consumption order: qk-weights + first xq slices, then
                # v weights + first xv slices, then the rest per tn.
                for j in range(CP):
                    nc.sync.dma_start(out=wqk_sb[j], in_=wqk.ap()[j])
                for j in range(CP):
                    nc.sync.dma_start(out=xq_sb[j][:, :, 0:TJ],
                                      in_=xq.ap()[j][:, :, 0:TJ])
                for c in range(CC):
                    csl = slice(c * 128, (c + 1) * 128)
                    nc.sync.dma_start(out=wv_sb[c], in_=wv.ap()[csl, :])
                    nc.sync.dma_start(out=xv_sb[c][:, 0:TJ],
                                      in_=xv.ap()[csl, 0:TJ])
                for tn in range(1, NTJ):
                    tsl = slice(tn * TJ, (tn + 1) * TJ)
                    for j in range(CP):
                        nc.sync.dma_start(out=xq_sb[j][:, :, tsl],
                                          in_=xq.ap()[j][:, :, tsl])
                    for c in range(CC):
                        csl = slice(c * 128, (c + 1) * 128)
                        nc.sync.dma_start(out=xv_sb[c][:, tsl],
                                          in_=xv.ap()[csl, tsl])

                # per tn: Q (fp8 DR), K (fp8 DR), V (bf16) so attention unit
                # (tj=tn, *) unblocks right after block tn.
                for tn in range(NTJ):
                    tsl = slice(tn * TJ, (tn + 1) * TJ)
                    for mg in range(HG // 2):
                        msl = slice(mg * 128, (mg + 1) * 128)
                        qp = pps.tile([128, TJ], F32, name="qp", tag="pp")
                        for j in range(CP):
                            nc.tensor.matmul(
                                qp, wqk_sb[j][:, :, msl], xq_sb[j][:, :, tsl],
                                start=(j == 0), stop=(j == CP - 1),
                                perf_mode=DR)
                        qs = qstage.tile([128, TJ], F8, name="qs")
                        nc.scalar.copy(qs, qp)
                        nc.sync.dma_start(
                            out=q_dram.ap()[msl, tsl], in_=qs)
                    for mg in range(HG // 2):
                        msl = slice(512 + mg * 128, 512 + (mg + 1) * 128)
                        kp = pps.tile([128, TJ], F32, name="kp", tag="pp")
                        for j in range(CP):
                            nc.tensor.matmul(
                                kp, wqk_sb[j][:, :, msl], xq_sb[j][:, :, tsl],
                                start=(j == 0), stop=(j == CP - 1),
                                perf_mode=DR)
                        nc.scalar.copy(k_sb[mg][:, tsl], kp)
                    for si in range(4 * tn, 4 * tn + 4):
                        ssl = slice(si * 128, (si + 1) * 128)
                        vp = pps.tile([128, HG * DH], F32, name="vp", tag="pp")
                        for c in range(CC):
                            nc.tensor.matmul(
                                vp, xv_sb[c][:, ssl], wv_sb[c],
                                start=(c == 0), stop=(c == CC - 1))
                        nc.vector.tensor_copy(
                            v_sb[si][:, :, 0:DH],
                            vp.rearrange("p (h d) -> p h d", h=HG))
                        nc.vector.tensor_copy(
                            out=v_sb[si][:, :, DH:DH + 1],
                            in_=ones_sb[:, :, None])

            # ---- phases 2+3 pools (reuse the released xw zone) ----
            with (
                tc.tile_pool(name="late", bufs=1) as late,
                tc.tile_pool(name="ppool", bufs=7) as ppool,
                tc.tile_pool(name="npool", bufs=4) as npool,
                tc.tile_pool(name="outpool", bufs=6) as outpool,
                tc.tile_pool(name="aps", bufs=2, space="PSUM") as aps,
            ):
                wp_sb = []
                for hp in range(HG // 2):
                    t_ = late.tile([128, C], BF16, name=f"wp_{hp}")
                    nc.sync.dma_start(out=t_,
                                      in_=wp.ap()[hp * 128:(hp + 1) * 128, :])
                    wp_sb.append(t_)
                o_sb = [late.tile([128, T], BF16, name=f"o_{hp}")
                        for hp in range(HG // 2)]

                # ---- phase 2: attention, two head-pair units interleaved ----
                def attn_unit_setup(hp, tj, par):
                    tsl = slice(tj * TJ, (tj + 1) * TJ)
                    q0, q1 = q0pad[par], q1pad[par]
                    nc.sync.dma_start(
                        out=q0[0:64, :],
                        in_=q_dram.ap()[hp * 128:hp * 128 + 64, tsl])
                    nc.sync.dma_start(
                        out=q1[64:128, :],
                        in_=q_dram.ap()[hp * 128 + 64:(hp + 1) * 128, tsl])
                    o_psA = aps.tile([DH + 1, TJ], F32, name="o_ps0", bufs=1)
                    o_psB = aps.tile([DH + 1, TJ], F32, name="o_ps1", bufs=1)
                    return (hp, q0, q1, o_psA, o_psB)

                def attn_chunk(unit, tj, si, nsi):
                    hp, q0, q1, o_ps0, o_ps1 = unit
                    h0, h1 = 2 * hp, 2 * hp + 1
                    r = si - 4 * tj
                    toff = 0 if r < 0 else 128 * r
                    ssl = slice(si * 128, (si + 1) * 128)

                    s_ps = aps.tile([128, 2, TJ], F32, name="s_ps", bufs=3)
                    nc.tensor.matmul(
                        s_ps[:, 0, toff:TJ],
                        k_sb[hp][:, ssl], q0[:, toff:TJ],
                        start=True, stop=True)
                    nc.tensor.matmul(
                        s_ps[:, 1, toff:TJ],
                        k_sb[hp][:, ssl], q1[:, toff:TJ],
                        start=True, stop=True)
                    if r >= 0:
                        nc.vector.tensor_tensor(
                            out=s_ps[:, :, toff:toff + 128],
                            in0=s_ps[:, :, toff:toff + 128],
                            in1=tril_sb[:, None, :].to_broadcast(
                                (128, 2, 128)),
                            op=mybir.AluOpType.add)
                    p_sb = ppool.tile([128, 2, TJ], BF16, name="p_sb")
                    nc.scalar.activation(
                        p_sb[:, :, toff:TJ], s_ps[:, :, toff:TJ],
                        mybir.ActivationFunctionType.Exp, scale=SCALE)
                    nc.tensor.matmul(
                        o_ps0[:, toff:TJ], v_sb[si][:, h0, :],
                        p_sb[:, 0, toff:TJ],
                        start=(si == 0), stop=(si == nsi - 1))
                    nc.tensor.matmul(
                        o_ps1[:, toff:TJ], v_sb[si][:, h1, :],
                        p_sb[:, 1, toff:TJ],
                        start=(si == 0), stop=(si == nsi - 1))

                def attn_norm(unit, tj):
                    # normalize: divide rows 0..63 by row 64 (L sums).
                    # Copy PSUM->SBUF first so the o_ps slot frees without
                    # waiting for the L DMA-broadcast roundtrip.
                    hp, q0, q1, o_ps0, o_ps1 = unit
                    tsl = slice(tj * TJ, (tj + 1) * TJ)
                    for idx, o_ps in ((0, o_ps0), (1, o_ps1)):
                        lrow = (hp * 2 + idx) * NTJ + tj
                        o_stage = npool.tile([DH + 1, TJ], F32,
                                             name="o_stage")
                        nc.vector.tensor_copy(o_stage, o_ps)
                        nc.sync.dma_start(
                            out=l_dram.ap()[lrow:lrow + 1, :],
                            in_=o_stage[DH:DH + 1, :])
                        lb = npool.tile([64, TJ], F32, name="lb")
                        nc.sync.dma_start(
                            out=lb,
                            in_=l_dram.ap()[lrow:lrow + 1, :]
                            .to_broadcast((64, TJ)))
                        linv = npool.tile([64, TJ], F32, name="linv")
                        nc.vector.reciprocal_approx_fast(linv, lb)
                        if idx == 0:
                            nc.gpsimd.tensor_tensor(
                                out=o_sb[hp][0:64, tsl],
                                in0=o_stage[0:DH, :],
                                in1=linv, op=mybir.AluOpType.mult)
                        else:
                            o_tmp = npool.tile([64, TJ], BF16, name="o_tmp")
                            nc.gpsimd.tensor_tensor(
                                out=o_tmp, in0=o_stage[0:DH, :],
                                in1=linv, op=mybir.AluOpType.mult)
                            nc.sync.dma_start(
                                out=o_sb[hp][64:128, tsl], in_=o_tmp)

                def proj_tile(ti, en):
                    tsl = slice(ti * 128, (ti + 1) * 128)
                    esl = slice(en * TJ, (en + 1) * TJ)
                    op_ps = aps.tile([128, TJ], F32, name="op_ps",
                                     tag="s_ps", bufs=3)
                    for hp in range(HG // 2):
                        nc.tensor.matmul(
                            op_ps, o_sb[hp][:, tsl], wp_sb[hp][:, esl],
                            start=(hp == 0), stop=(hp == HG // 2 - 1))
                    ob = outpool.tile([128, TJ], BF16, name="ob")
                    nc.vector.tensor_copy(ob, op_ps)
                    nc.sync.dma_start(out=out.ap()[tsl, esl], in_=ob)

                # tj-major so tj's output projection overlaps tj+1 attention.
                for tj in range(NTJ):
                    for hp in range(HG // 2):
                        unit = attn_unit_setup(hp, tj, (tj * (HG // 2) + hp) % 2)
                        nsi = 4 * tj + 4
                        for si in range(nsi):
                            attn_chunk(unit, tj, si, nsi)
                        attn_norm(unit, tj)
                    for ti in range(4 * tj, 4 * tj + 4):
                        for en in range(C // TJ):
                            proj_tile(ti, en)

    nc.compile()
    return nc


def _get_nc():
    if "nc" not in _NC_CACHE:
        _NC_CACHE["nc"] = _build()
    return _NC_CACHE["nc"]


def _pair_rows(a):
    """[C, N] -> [CP, 128, 2, N] DoubleRow pairing: (j, p, i) = row
    256*j + 128*i + p."""
    n = a.shape[1]
    return np.ascontiguousarray(
        a.reshape(CP, 2, 128, n).transpose(0, 2, 1, 3))


def _make_in_maps(x, Wq, Wk, Wv, Wp):
    tril_h = np.where(
        np.arange(128)[:, None] > np.arange(128)[None, :],
        np.float32(MASK_VAL), np.float32(0.0)).astype(np.float32)
    in_maps = []
    for core in range(8):
        b, g = core // 2, core % 2
        heads = range(g * HG, (g + 1) * HG)
        wq = np.concatenate([Wq[h] for h in heads], axis=1) * W_SCALE
        wk = np.concatenate([Wk[h] for h in heads], axis=1) * W_SCALE
        wv_ = np.concatenate([Wv[h] for h in heads], axis=1)
        xt = np.ascontiguousarray(x[b].T)
        in_maps.append({
            "xq": _pair_rows(xt).astype(NP_F8),
            "xv": xt.astype(NP_BF16),
            "wqk": _pair_rows(
                np.concatenate([wq, wk], axis=1)).astype(NP_F8),
            "wv": wv_.astype(NP_BF16),
            "wp": Wp[g * HG * DH:(g + 1) * HG * DH, :].astype(NP_BF16),
            "tril": tril_h,
            "ones8": np.ones((128, HG), NP_BF16),
            "zrow": np.zeros((1, TJ), NP_F8),
        })
    return in_maps


_LAST_RESULTS = {}


def kernel(x, Wq, Wk, Wv, Wp, bp):
    x = np.asarray(x, np.float32)
    Wq = np.asarray(Wq, np.float32)
    Wk = np.asarray(Wk, np.float32)
    Wv = np.asarray(Wv, np.float32)
    Wp = np.asarray(Wp, np.float32)
    bp = np.asarray(bp, np.float32)

    nc = _get_nc()
    in_maps = _make_in_maps(x, Wq, Wk, Wv, Wp)
    res = bass_utils.run_bass_kernel_spmd(
        nc, in_maps, core_ids=list(range(8)), trace=TRACE)
    _LAST_RESULTS["res"] = res

    out = np.empty((B, T, C), np.float32)
    for b in range(B):
        out[b] = (res.results[2 * b]["out"].astype(np.float32)
                  + res.results[2 * b + 1]["out"].astype(np.float32) + bp)
    return out
